# revision 16
# baseline (speedup 1.0000x reference)
"""Trainium2 Bass kernel for nn_BlockUngrouper.

Problem: out[b, n, :] = block_features[b, g, k, :] where g is the block whose
one-hot claims token n and k is n's rank within that block (cumsum of the
one-hot along n).  The input distribution (per-sample permutation partition)
guarantees each token is claimed by exactly one block and ranks < 128, so per
batch this is a row-permutation gather with
    flat_idx[n] = 128 * g(n) + rank(n).

Sharding: data-parallel over the batch dim, 2 batches per NeuronCore x 8.

Per-core program (all index arithmetic exact in fp32):
  1. onehot [N, 128] -> SBUF f32 chunks (HWDGE), ACT-engine copy casts to
     bf16 in layout [token-in-tile, (tile, g)].
  2. counts[g, t]: per 128-token tile, PE matmul lhsT=OH rhs=ones.
  3. incl/pex[g, t]: scan over tiles (DVE tensor_tensor_scan), add
     128*g - 1 + b*N (gmat), PE-transpose to [t, g], flatten 16-tile slices
     to partition-0 rows (small HWDGE SBUF->SBUF DMA).
  4. per 4-tile PSUM group: broadcast-add prefix via K=1 matmul (start),
     then 4 upper-triangular matmuls add the within-tile inclusive cumsum;
     DVE scalar_tensor_tensor (PSUM x onehot, accum_out over g) selects each
     token's entry -> flat_mat[p, t] = feat row index (b*N included).
  5. idx shuffle: 8 selector matmuls (Rall) reshape flat_mat [p, t] into the
     dma_gather index layout (idx j at partition j%16, col j//16, replicated
     x8 across partition groups), one DVE copy casts PSUM->int16 with an
     interleaving access pattern.
  6. per 16-tile chunk (2048 tokens): ONE bulk SWDGE dma_gather pulls 2048 x
     1KB feature rows into SBUF (2 MB), then one HWDGE store DMA writes them
     to out.  8 gathers + 8 stores per batch vs 256 indirect DMAs in v7 --
     SWDGE descriptor-generation cost drops from ~256us to ~27us per core.
"""

from contextlib import ExitStack

import numpy as np

import concourse.bass as bass
import concourse.bacc as bacc
import concourse.mybir as mybir
import concourse.tile as tile
from concourse import bass_utils
from concourse.masks import make_identity, make_upper_triangular
from concourse import library_config

P = 128  # partitions = tokens per tile = G (blocks) = NG_MAX
KERNEL_VERSION = 9  # bump on every meaningful kernel change (NEFF-cache buster)
N_CORES = 8
B_FULL = 16  # full batch dim
N_TOK = 16384  # tokens per batch
D_FEAT = 256  # feature dim
NB = B_FULL // N_CORES  # batches per core

FP32 = mybir.dt.float32
BF16 = mybir.dt.bfloat16
I16 = mybir.dt.int16
I32 = mybir.dt.int32


def build_nc(NB: int, N: int, D: int, CT: int = 8, GRP: int = 4, LC: int = 16,
             STG_BUFS: int = 2, IDX_BUFS: int = 2, REPS: int = 1,
             DYN_LOOP: int = 0, MODE: str = "full", SCRATCH: int = 65536,
             TIMING: bool = False, FEAT_ROWS: int = 0, SP: int = 1):
    """Build the per-core bass program.

    NB: batches per core; N: tokens per batch; D: feature dim.
    CT: tiles per gather/store chunk; GRP: tiles per PSUM group; LC: tiles
    per onehot load chunk.
    TIMING: features/out become Internal DRAM tensors (no per-call host
    transfer; gather addresses depend only on onehot, values don't matter),
    and a tiny dummy output keeps the NEFF valid.  Timing-only builds.
    """
    T = N // P  # token tiles per batch
    assert T * P == N
    GRP = min(GRP, T)
    CT = min(CT, T)
    LC = min(LC, T)
    assert T % GRP == 0 and T % CT == 0 and T % LC == 0
    NI = CT * P  # tokens (gather indices) per chunk
    add = mybir.AluOpType.add
    mult = mybir.AluOpType.mult
    bypass = mybir.AluOpType.bypass

    nc = bacc.Bacc("TRN2", target_bir_lowering=False, debug=False,
                   dynamic_dma_scratch_size=SCRATCH)

    io_kind = "Internal" if TIMING else None
    FEAT_ROWS = FEAT_ROWS or N  # feat rows per batch (128 * G in general)
    feat = nc.dram_tensor("block_features", [NB * FEAT_ROWS, D], FP32,
                          kind=io_kind or "ExternalInput")
    oh = nc.dram_tensor("block_onehot", [NB, N, P], FP32, kind="ExternalInput")
    out = nc.dram_tensor("out", [NB, N, D], FP32, kind=io_kind or "ExternalOutput")
    if TIMING:
        nc.dram_tensor("dummy_out", [1, 8], FP32, kind="ExternalOutput")
    # The PJRT NEFF cache keys on the HLO alone (the embedded bass program
    # does not enter the hash), so distinct kernel versions collide.  A dummy
    # input whose shape encodes a version nonce forces a distinct hash.
    import zlib as _zlib
    _nonce = (
        _zlib.crc32(
            f"v{KERNEL_VERSION}-{NB}-{N}-{D}-{CT}-{GRP}-{LC}-{STG_BUFS}-{IDX_BUFS}-{REPS}-{DYN_LOOP}-{MODE}-{SCRATCH}-{TIMING}-{FEAT_ROWS}-{SP}".encode()
        )
        % 4093
        + 1
    )
    nc.dram_tensor("version_tag", [1, _nonce], FP32, kind="ExternalInput")

    with tile.TileContext(nc) as tc, ExitStack() as ctx:
        cpool = ctx.enter_context(tc.tile_pool(name="const", bufs=1))
        ohpool = ctx.enter_context(tc.tile_pool(name="ohp", bufs=2))
        ldpool = ctx.enter_context(tc.tile_pool(name="ld", bufs=2))
        wpool = ctx.enter_context(tc.tile_pool(name="work", bufs=2))
        ppool = ctx.enter_context(tc.tile_pool(name="psum", bufs=2, space="PSUM"))
        pspool = ctx.enter_context(tc.tile_pool(name="psumsm", bufs=2, space="PSUM"))
        spool = ctx.enter_context(tc.tile_pool(name="stage", bufs=STG_BUFS))
        fpool = ctx.enter_context(tc.tile_pool(name="flat", bufs=2))
        xpool = ctx.enter_context(tc.tile_pool(name="xrep", bufs=IDX_BUFS))

        # --- constants ---
        triu = cpool.tile([P, P], BF16)  # triu[k, m] = 1 iff k <= m
        make_upper_triangular(nc, triu[:], val=1.0, diag=True)
        ident = cpool.tile([P, P], FP32)
        make_identity(nc, ident[:])
        ones_col = cpool.tile([P, 1], BF16)
        nc.gpsimd.memset(ones_col[:], 1.0)
        ones_row = cpool.tile([1, P], FP32)
        nc.gpsimd.memset(ones_row[:], 1.0)
        # gmat_b[g, t] = 128*g - 1 + b*N (constant along t)
        gmats = []
        for b in range(NB):
            gi = cpool.tile([P, T], I32, tag=f"gi{b}")
            nc.gpsimd.iota(gi[:], pattern=[[0, T]], base=b * FEAT_ROWS - 1,
                           channel_multiplier=P)
            gf = cpool.tile([P, T], FP32, tag=f"gmat{b}")
            nc.vector.tensor_copy(gf[:], gi[:])
            gmats.append(gf)
        # Rall[p, m*128 + p'] = 1 iff p == 16*m + p'%16  (selector blocks for
        # the dma_gather index shuffle: block m broadcasts flat_mat rows
        # 16m..16m+15 to every 16-partition group)
        e_i = cpool.tile([P, 8 * P], I32)
        nc.gpsimd.iota(e_i[:], pattern=[[16, 8], [0, 8], [1, 16]], base=0,
                       channel_multiplier=0)
        p_i = cpool.tile([P, 8 * P], I32)
        nc.gpsimd.iota(p_i[:], pattern=[[0, 8 * P]], base=0,
                       channel_multiplier=1)
        rall = cpool.tile([P, 8 * P], FP32)
        nc.vector.tensor_tensor(out=rall[:], in0=e_i[:], in1=p_i[:],
                                op=mybir.AluOpType.is_equal)
        fiota = []
        if MODE == "gather":
            for b in range(NB):
                fi_i = cpool.tile([P, T], I32)
                nc.gpsimd.iota(fi_i[:], pattern=[[P, T]], base=b * N,
                               channel_multiplier=1)
                ff = cpool.tile([P, T], FP32)
                nc.vector.tensor_copy(ff[:], fi_i[:])
                fiota.append(ff)

        import contextlib
        loop_cm = tc.For_i(0, DYN_LOOP, 1) if DYN_LOOP else contextlib.nullcontext()
        with loop_cm:
          for rep in range(REPS):
            for b in range(NB):
                oh_src = oh.ap()[b].rearrange("(t p) g -> p t g", p=P)
                out_dst = out.ap()[b].rearrange("(t p) d -> p t d", p=P)

                if MODE == "gather":
                    # Diagnostic: iota indices (from constants) through the
                    # real idx-shuffle machinery, then gathers + stores only.
                    flat_mat = fiota[b]
                else:
                    flat_mat = wpool.tile([P, T], FP32, tag="flatmat")
                    oh_sb = ohpool.tile([P, T * P], BF16, tag="oh")
                    countsT_ps = pspool.tile([P, T], FP32, tag="counts")
                    incl = wpool.tile([P, T], FP32, tag="incl")  # [g, t] incl
                    pex_adj = wpool.tile([P, T], FP32, tag="pexadj")
                    padjT = wpool.tile([T, P], FP32, tag="padjT")  # [t, g]

                    # --- load + cast + counts + scan per LC chunk ---
                    for lc in range(T // LC):
                        lc0, lc1 = lc * LC, (lc + 1) * LC
                        ld = ldpool.tile([P, LC * P], FP32, tag="ld")
                        nc.sync.dma_start(out=ld[:], in_=oh_src[:, lc0:lc1, :])
                        nc.scalar.copy(oh_sb[:, lc0 * P: lc1 * P], ld[:])
                        for t in range(lc0, lc1):
                            nc.tensor.matmul(
                                out=countsT_ps[:, t: t + 1],
                                lhsT=oh_sb[:, t * P: (t + 1) * P],
                                rhs=ones_col[:],
                                start=True,
                                stop=True,
                            )
                        nc.vector.tensor_tensor_scan(
                            out=incl[:, lc0:lc1],
                            data0=countsT_ps[:, lc0:lc1],
                            data1=gmats[b][:, 0:LC],
                            initial=(0.0 if lc == 0 else incl[:, lc0 - 1: lc0]),
                            op0=add,
                            op1=bypass,
                        )
                    # --- prefix: pex_adj = incl - counts + gmat; transpose;
                    #     flatten slices to partition-0 rows ---
                    nc.vector.tensor_tensor(
                        out=pex_adj[:], in0=incl[:], in1=countsT_ps[:],
                        op=mybir.AluOpType.subtract,
                    )
                    nc.vector.tensor_tensor(
                        out=pex_adj[:], in0=pex_adj[:], in1=gmats[b][:],
                        op=add,
                    )
                    padjT_ps = pspool.tile([T, P], FP32, tag="padjT_ps")
                    nc.tensor.transpose(
                        out=padjT_ps[:], in_=pex_adj[:], identity=ident[:]
                    )
                    nc.vector.tensor_copy(padjT[:], padjT_ps[:])
                    SL = 8  # tiles per flatten slice
                    for s in range(T // SL):
                        flat_row = fpool.tile([1, SL * P], FP32, tag="flatrow")
                        nc.scalar.dma_start(
                            out=flat_row[:],
                            in_=padjT[s * SL: (s + 1) * SL, :],
                        )
                        # --- groups: prefix bcast + within-tile cumsum + select
                        for grp in range(s * SL // GRP, (s + 1) * SL // GRP):
                            g_in_s = grp - s * SL // GRP
                            grp_ps = ppool.tile([P, GRP * P], FP32, tag="grp")
                            nc.tensor.matmul(
                                out=grp_ps[:],
                                lhsT=ones_row[:],
                                rhs=flat_row[0:1, g_in_s * GRP * P:
                                             (g_in_s + 1) * GRP * P],
                                start=True,
                                stop=False,
                                skip_group_check=True,
                            )
                            for i in range(GRP):
                                t = grp * GRP + i
                                nc.tensor.matmul(
                                    out=grp_ps[:, i * P: (i + 1) * P],
                                    lhsT=triu[:],
                                    rhs=oh_sb[:, t * P: (t + 1) * P],
                                    start=False,
                                    stop=True,
                                    skip_group_check=True,
                                )
                            scratch = wpool.tile([P, GRP * P], FP32, tag="scr")
                            for i in range(GRP):
                                t = grp * GRP + i
                                nc.vector.scalar_tensor_tensor(
                                    out=scratch[:, i * P: (i + 1) * P],
                                    in0=grp_ps[:, i * P: (i + 1) * P],
                                    scalar=1.0,
                                    in1=oh_sb[:, t * P: (t + 1) * P],
                                    op0=mult,
                                    op1=mult,
                                    accum_out=flat_mat[:, t: t + 1],
                                )

                if MODE == "index":
                    nc.sync.dma_start(
                        out=out_dst[:, 0:1, 0:T], in_=flat_mat[:]
                    )
                    continue

                # --- idx shuffle: flat_mat [p, t] -> dma_gather layout ---
                xrep = xpool.tile([P, 8 * T], I16, tag="xrep")
                for h in range(2):
                    rep_ps = ppool.tile([P, 4 * T], FP32, tag="grp")
                    for mm in range(4):
                        m = h * 4 + mm
                        nc.tensor.matmul(
                            out=rep_ps[:, mm * T: (mm + 1) * T],
                            lhsT=rall[:, m * P: (m + 1) * P],
                            rhs=flat_mat[:],
                            start=True,
                            stop=True,
                        )
                    # xrep[q', t*8 + h*4 + mm] = rep_ps[q', mm*T + t]
                    nc.vector.tensor_copy(
                        xrep[:].rearrange("p (t m) -> p t m", m=8)[:, :, h * 4: (h + 1) * 4],
                        rep_ps[:].rearrange("p (m t) -> p t m", t=T),
                    )

                # --- bulk gathers + stores ---
                for c in range(T // CT):
                    stg = spool.tile([P, CT * D], FP32, tag="stg")
                    nc.gpsimd.dma_gather(
                        stg[:].rearrange("p (t d) -> p t d", d=D),
                        feat.ap(),
                        xrep[:, c * CT * 8: (c + 1) * CT * 8],
                        NI,
                        NI,
                        D,
                        single_packet=bool(SP),
                    )
                    nc.sync.dma_start(
                        out=out_dst[:, c * CT: (c + 1) * CT, :], in_=stg[:]
                    )

    nc.compile()
    return nc


_NC_CACHE = {}


def _get_nc():
    key = (NB, N_TOK, D_FEAT)
    if key not in _NC_CACHE:
        _NC_CACHE[key] = build_nc(*key)
    return _NC_CACHE[key]


def make_in_maps(block_features: np.ndarray, block_onehot: np.ndarray):
    """Shard full inputs batch-wise into 8 per-core input maps."""
    feat = np.ascontiguousarray(block_features, dtype=np.float32).reshape(
        B_FULL, N_TOK, D_FEAT
    )
    oh = np.ascontiguousarray(block_onehot, dtype=np.float32)
    nc = _get_nc()
    tag_shape = None
    for alloc in nc.m.functions[0].allocations:
        if isinstance(alloc, mybir.MemoryLocationSet) and alloc.kind == "ExternalInput":
            if alloc.memorylocations[0].name == "version_tag":
                tag_shape = tuple(alloc.tensor_shape)
    in_maps = []
    for c in range(N_CORES):
        lo, hi = c * NB, (c + 1) * NB
        m = {
            "block_features": feat[lo:hi].reshape(NB * N_TOK, D_FEAT),
            "block_onehot": oh[lo:hi],
        }
        if tag_shape is not None:
            m["version_tag"] = np.zeros(tag_shape, np.float32)
        in_maps.append(m)
    return in_maps


def run_spmd(in_maps, trace: bool = False):
    """Compile (cached) + run the SPMD program on cores 0-7."""
    nc = _get_nc()
    return bass_utils.run_bass_kernel_spmd(
        nc, in_maps, core_ids=list(range(N_CORES)), trace=trace
    )


def kernel(**inputs) -> np.ndarray:
    block_features = inputs["block_features"]
    block_onehot = inputs["block_onehot"]
    in_maps = make_in_maps(block_features, block_onehot)
    res = run_spmd(in_maps, trace=False)
    out = np.concatenate([r["out"] for r in res.results], axis=0)
    return out.reshape(B_FULL, N_TOK, D_FEAT)
